# revision 10
# baseline (speedup 1.0000x reference)
"""GPTQ 4-bit quantized linear (GPTQLinear_woo) on 8 Trainium2 NeuronCores.

Column-parallel: qweight/scales/zeros sharded along out-features (11008/8 =
1376 per core), x replicated. Per core the device kernel:
  - unpacks nibbles from the uint16 view of qweight with tensor_scalar
    (shift+and) straight to bf16,
  - multiplies by per-group scales (pre-broadcast to all 128 in-feature rows
    of a tile via a tiny one-hot matmul on TensorE),
  - accumulates x @ (s*q) with bf16 matmuls into f32 PSUM,
  - folds the zeros term in as one K=32 matmul per output tile:
      out -= Sx @ zeros, Sx[b,g] = sum of x over group g.
Host side only reshapes/slices/replicates (no arithmetic on input data).
"""

import sys

if "/opt/trn_rl_repo" not in sys.path:
    sys.path.insert(0, "/opt/trn_rl_repo")

from contextlib import ExitStack

import numpy as np

import concourse.bacc as bacc
import concourse.bass as bass
import concourse.mybir as mybir
import concourse.tile as tile
from concourse.bass_utils import run_bass_kernel_spmd

dt = mybir.dt
Alu = mybir.AluOpType

N_CORES = 8
BATCH = 128
IN_F = 4096
OUT_F = 11008
GS = 128
NG = IN_F // GS  # 32
RROWS = IN_F // 8  # 512 packed rows
OC = OUT_F // N_CORES  # 1376 out-features per core
OC2 = 2 * OC  # 2752 uint16 columns per core
NRT = RROWS // 128  # 4 row tiles
OTS = [(o0, min(512, OC - o0)) for o0 in range(0, OC, 512)]  # [(0,512),(512,512),(1024,352)]

_CACHE = {}


def _build_program():
    nc = bacc.Bacc("TRN2", target_bir_lowering=False, debug=False, num_devices=N_CORES)

    qw = nc.dram_tensor("qw", [RROWS, OC2], dt.uint16, kind="ExternalInput").ap()
    xr = nc.dram_tensor("xr", [128, 32 * 128], dt.float32, kind="ExternalInput").ap()
    sc2 = nc.dram_tensor("sc2", [NG, OC2], dt.float32, kind="ExternalInput").ap()
    zr = nc.dram_tensor("zr", [NG, OC], dt.float32, kind="ExternalInput").ap()
    bg = nc.dram_tensor("bg", [128, 8], dt.bfloat16, kind="ExternalInput").ap()
    b32 = nc.dram_tensor("b32", [NG, 4 * 128], dt.bfloat16, kind="ExternalInput").ap()
    idn = nc.dram_tensor("idn", [128, 128], dt.float32, kind="ExternalInput").ap()
    out = nc.dram_tensor("out", [BATCH, OC], dt.float32, kind="ExternalOutput").ap()

    with tile.TileContext(nc) as tc, ExitStack() as ctx:
        const = ctx.enter_context(tc.tile_pool(name="const", bufs=1))
        qwp = ctx.enter_context(tc.tile_pool(name="qwp", bufs=2))
        nibp = ctx.enter_context(tc.tile_pool(name="nibp", bufs=3))
        wtp = ctx.enter_context(tc.tile_pool(name="wtp", bufs=3))
        psum = ctx.enter_context(tc.tile_pool(name="psum", bufs=1, space="PSUM"))
        pscb = ctx.enter_context(tc.tile_pool(name="pscb", bufs=2, space="PSUM"))

        # ---- constant / small loads ----
        bg_sb = const.tile([128, 8], dt.bfloat16)
        nc.sync.dma_start(bg_sb[:], bg[:])
        b32_sb = const.tile([NG, 4 * 128], dt.bfloat16)
        nc.sync.dma_start(b32_sb[:], b32[:])
        idn_sb = const.tile([128, 128], dt.float32)
        nc.sync.dma_start(idn_sb[:], idn[:])
        sc2_sb = const.tile([NG, OC2], dt.float32)
        nc.sync.dma_start(sc2_sb[:], sc2[:])
        zr_sb = const.tile([NG, OC], dt.float32)
        nc.sync.dma_start(zr_sb[:], zr[:])
        xr_f = const.tile([128, 32 * 128], dt.float32)
        nc.sync.dma_start(xr_f[:], xr[:])

        # casts (off the Vector engine: GpSimd + Scalar)
        xr_bf = const.tile([128, 32 * 128], dt.bfloat16)
        nc.gpsimd.tensor_copy(xr_bf[:], xr_f[:])
        sc2_bf = const.tile([NG, OC2], dt.bfloat16)
        nc.scalar.copy(sc2_bf[:], sc2_sb[:])
        nzr_bf = const.tile([NG, OC], dt.bfloat16)
        nc.scalar.mul(nzr_bf[:], zr_sb[:], -1.0)

        # ---- scales broadcast: scexp[rt][rl, c] = sc2[8*rt + rl//16, c] (bf16)
        scexp = []
        for rt in range(NRT):
            se = const.tile([128, OC2], dt.bfloat16, tag=f"scexp{rt}")
            for c0 in range(0, OC2, 512):
                cn = min(512, OC2 - c0)
                ps = pscb.tile([128, 512], dt.float32, tag="pscb")
                nc.tensor.matmul(
                    ps[:, :cn], b32_sb[:, 128 * rt : 128 * (rt + 1)], sc2_bf[:, c0 : c0 + cn],
                    start=True, stop=True,
                )
                nc.scalar.copy(se[:, c0 : c0 + cn], ps[:, :cn])
            scexp.append(se)

        # ---- Sx[b, g] = sum over group g of x[b, :]  (psum, then transpose)
        ps_sx = psum.tile([128, NG], dt.float32, tag="ps_sx")
        for rt in range(NRT):
            for s in range(8):
                t = s * 4 + rt
                nc.tensor.matmul(
                    ps_sx[:, 8 * rt : 8 * rt + 8],
                    xr_bf[:, 128 * t : 128 * (t + 1)],
                    bg_sb[:],
                    start=(s == 0), stop=(s == 7),
                )
        sx_sb = const.tile([128, NG], dt.float32)
        nc.scalar.copy(sx_sb[:], ps_sx[:])
        ps_sxT = psum.tile([NG, 128], dt.float32, tag="ps_sxT")
        nc.tensor.transpose(ps_sxT[:], sx_sb[:], idn_sb[:])
        sxT_bf = const.tile([NG, 128], dt.bfloat16)
        nc.scalar.copy(sxT_bf[:], ps_sxT[:])

        # ---- main accumulation
        psum_out = [
            psum.tile([128, 512], dt.float32, tag=f"po{ot}", name=f"po{ot}")
            for ot in range(len(OTS))
        ]
        # zeros term first (start=True clears the bank)
        for ot, (o0, n) in enumerate(OTS):
            nc.tensor.matmul(
                psum_out[ot][:, :n], sxT_bf[:], nzr_bf[:, o0 : o0 + n],
                start=True, stop=False,
            )

        for rt in range(NRT):
            qwt = qwp.tile([128, OC2], dt.uint16, tag="qwt")
            nc.sync.dma_start(qwt[:], qw[128 * rt : 128 * (rt + 1), :])
            for k in range(4):
                nib = nibp.tile([128, OC2], dt.uint16, tag="nib")
                nc.vector.tensor_scalar(
                    nib[:], qwt[:], 4 * k, 15,
                    Alu.logical_shift_right, Alu.bitwise_and,
                )
                wt = wtp.tile([128, OC2], dt.bfloat16, tag="wt")
                nc.vector.tensor_tensor(wt[:], nib[:], scexp[rt][:], Alu.mult)
                for par, s in ((0, k), (1, k + 4)):
                    t = s * 4 + rt
                    lhsT = xr_bf[:, 128 * t : 128 * (t + 1)]
                    for ot, (o0, n) in enumerate(OTS):
                        last = rt == NRT - 1 and k == 3 and par == 1
                        nc.tensor.matmul(
                            psum_out[ot][:, :n],
                            lhsT,
                            wt[:, par + 2 * o0 : par + 2 * (o0 + n) - 1 : 2],
                            start=False, stop=last,
                        )

        out_sb = const.tile([BATCH, OC], dt.float32)
        for ot, (o0, n) in enumerate(OTS):
            nc.scalar.copy(out_sb[:, o0 : o0 + n], psum_out[ot][:, :n])
        nc.sync.dma_start(out[:], out_sb[:])

    nc.compile()
    return nc


def _host_inputs(x, qweight, scales, zeros):
    """Pure relayout/replication — per-core input maps."""
    x = np.asarray(x, dtype=np.float32)
    qweight = np.ascontiguousarray(np.asarray(qweight, dtype=np.int32))
    scales = np.asarray(scales, dtype=np.float32)
    zeros = np.asarray(zeros, dtype=np.float32)

    # xr[rl, t*128 + b] = x[b, 8*(128*rt + rl) + s], t = s*4 + rt
    xrh = np.ascontiguousarray(
        x.reshape(BATCH, NRT, 128, 8).transpose(2, 3, 1, 0).reshape(128, 8 * NRT * BATCH)
    )
    # (rl, s, rt, b) -> free index (s*4+rt)*128 + b  ✓

    bg_np = np.zeros((128, 8), np.float32)
    bg_np[np.arange(128), np.arange(128) // 16] = 1.0
    b32_np = np.zeros((NG, 4 * 128), np.float32)
    for rt in range(4):
        for p in range(128):
            b32_np[8 * rt + p // 16, 128 * rt + p] = 1.0
    import ml_dtypes

    bg_bf = bg_np.astype(ml_dtypes.bfloat16)
    b32_bf = b32_np.astype(ml_dtypes.bfloat16)
    idn_np = np.eye(128, dtype=np.float32)

    in_maps = []
    for c in range(N_CORES):
        o0, o1 = c * OC, (c + 1) * OC
        qw_c = np.ascontiguousarray(qweight[:, o0:o1]).view(np.uint16)
        sc_c = np.repeat(scales[:, o0:o1], 2, axis=1)
        zr_c = np.ascontiguousarray(zeros[:, o0:o1])
        in_maps.append(
            {
                "qw": qw_c,
                "xr": xrh,
                "sc2": np.ascontiguousarray(sc_c),
                "zr": zr_c,
                "bg": bg_bf,
                "b32": b32_bf,
                "idn": idn_np,
            }
        )
    return in_maps


def kernel(x, qweight, scales, zeros, _trace=False):
    if "nc" not in _CACHE:
        _CACHE["nc"] = _build_program()
    nc = _CACHE["nc"]
    in_maps = _host_inputs(x, qweight, scales, zeros)
    res = run_bass_kernel_spmd(nc, in_maps, list(range(N_CORES)), trace=_trace)
    out = np.concatenate([res.results[c]["out"] for c in range(N_CORES)], axis=1)
    if _trace:
        _CACHE["last_results"] = res
    return np.ascontiguousarray(out.astype(np.float32))
